# revision 1
# baseline (speedup 1.0000x reference)
"""DependencyLabelClassifier pairwise-logits kernel for 8 Trainium2 NeuronCores.

Reference computation (B=32, L=128, D=384, NL=64):
  head[b,k,n] = emb[b,k,:] @ W[n,:D];  dep[b,j,n] = emb[b,j,:] @ W[n,D:]
  out[b, j*L+k, n] = head[b,k,n] + dep[b,j,n]  where att[b,j] & att[b,k] & j!=k
                   = -inf elsewhere

Sharding: data-parallel over batch, 4 batches per core. Per batch the
[128(j), 8192(k*64+n)] output tile keeps j on partitions so the 4MB DRAM
store is fully contiguous per partition (32KB descriptors, near-peak HBM
bandwidth) -- the kernel is output-bandwidth bound (134MB total).

Per-core pipeline per batch:
  1. head/dep via fp32 TensorE matmuls (contract D in 3 chunks of 128).
  2. head (+ -inf column mask) replicated across all 128 partitions with a
     contraction-3 bf16 ones-matmul over the exact hi/lo/lo2 bf16 split of
     head (error ~2^-24; bf16 carries -inf exactly). The [1,8192] flattened
     rows reach SBUF partition rows via a small DRAM round-trip.
  3. DVE adds dep (+ -inf row mask, folded in by ScalarE bias-add) with a
     stride-0 access pattern broadcasting dep[j,:] across the 32 k-groups
     of each PSUM chunk.
  4. 4MB contiguous store; then the j==k diagonal is overwritten with -inf
     by a [(L+1)*NL]-strided DRAM DMA.
"""
import numpy as np

import concourse.bass as bass
import concourse.mybir as mybir
from concourse.bass import AP
from concourse.tile import TileContext
from concourse import bass_utils

F32 = mybir.dt.float32
BF16 = mybir.dt.bfloat16
NEG_INF = float("-inf")

B, L, D, NL = 32, 128, 384, 64
NCORES = 8
NB = B // NCORES          # batches per core
LNL = L * NL              # 8192


def _split_excess_waits(nc, max_waits=1):
    """The walrus in this container rejects instructions carrying more than
    one semaphore wait ('Too many sync wait commands'). Move extras onto
    same-engine NoOps placed immediately before the offending instruction."""
    for f in nc.m.functions:
        for bb in f.blocks:
            new = []
            for inst in bb.instructions:
                si = inst.sync_info
                if si is not None and si.on_wait and len(si.on_wait) > max_waits:
                    waits = list(si.on_wait)
                    head, tail = waits[:-max_waits], waits[-max_waits:]
                    for i in range(0, len(head), max_waits):
                        new.append(mybir.InstNoOp(
                            name=f"{inst.name}_wsplit_{i}",
                            engine=inst.engine,
                            bass_nofuse=True,
                            ins=[], outs=[],
                            sync_info=mybir.SyncInfo(
                                on_wait=head[i:i + max_waits], on_update=[]),
                        ))
                    inst.sync_info = mybir.SyncInfo(
                        on_wait=tail, on_update=list(si.on_update or []))
                new.append(inst)
            bb.instructions = new


def build_nc(nb: int = NB, repeat: int = 1) -> bass.Bass:
    nc = bass.Bass()
    embT_d = nc.dram_tensor("embT", [nb, D, L], F32, kind="ExternalInput")
    w1T_d = nc.dram_tensor("w1T", [D, NL], F32, kind="ExternalInput")
    w2T_d = nc.dram_tensor("w2T", [D, NL], F32, kind="ExternalInput")
    negm_d = nc.dram_tensor("negm", [nb, L, 1], F32, kind="ExternalInput")
    out_d = nc.dram_tensor("out", [nb, L * L, NL], F32, kind="ExternalOutput")
    scr_d = nc.dram_tensor("scr", [nb, 3, LNL], BF16, kind="Internal")

    with TileContext(nc) as tc:
        with tc.tile_pool(name="const", bufs=1) as cpool, \
             tc.tile_pool(name="embT", bufs=2) as epool, \
             tc.tile_pool(name="small", bufs=3) as spool, \
             tc.tile_pool(name="hl", bufs=2) as hlpool, \
             tc.tile_pool(name="outp", bufs=2) as opool, \
             tc.tile_pool(name="ps", bufs=2, space="PSUM") as pspool:

            w1T = cpool.tile([128, 3 * NL], F32, tag="w1")
            w2T = cpool.tile([128, 3 * NL], F32, tag="w2")
            for w_t, w_d in ((w1T, w1T_d), (w2T, w2T_d)):
                nc.sync.dma_start(
                    w_t[:, :],
                    AP(w_d[:, :].tensor, 0, [[NL, 128], [128 * NL, 3], [1, NL]]))
            ones3_f = cpool.tile([3, 128], F32, tag="o3f")
            nc.vector.memset(ones3_f[:, :], 1.0)
            ones3 = cpool.tile([3, 128], BF16, tag="o3")
            nc.vector.tensor_copy(ones3[:, :], ones3_f[:, :])
            ninf = cpool.tile([128, NL], F32, tag="ninf")
            nc.vector.memset(ninf[:, :], NEG_INF)

            for _rep in range(repeat):
                for b in range(nb):
                    embT = epool.tile([128, D], F32, tag="embT")
                    nc.sync.dma_start(
                        embT[:, :],
                        AP(embT_d[:, :, :].tensor, b * D * L,
                           [[L, 128], [128 * L, 3], [1, L]]))
                    negm = spool.tile([128, 1], F32, tag="negm")
                    nc.sync.dma_start(negm[:, :], negm_d[b, :, :])

                    t_hd = pspool.tile([128, 2048], F32, tag="ps")
                    for c in range(3):
                        nc.tensor.matmul(t_hd[:, 0:NL],
                                         embT[:, c * 128:(c + 1) * 128],
                                         w1T[:, c * NL:(c + 1) * NL],
                                         start=(c == 0), stop=(c == 2))
                    for c in range(3):
                        nc.tensor.matmul(t_hd[:, NL:2 * NL],
                                         embT[:, c * 128:(c + 1) * 128],
                                         w2T[:, c * NL:(c + 1) * NL],
                                         start=(c == 0), stop=(c == 2))

                    dep = spool.tile([128, NL], F32, tag="dep")
                    nc.scalar.activation(dep[:, :], t_hd[:, NL:2 * NL],
                                         mybir.ActivationFunctionType.Identity,
                                         bias=negm[:, :])

                    hcl = spool.tile([128, NL], F32, tag="hcl")
                    nc.vector.tensor_copy(hcl[:, :], t_hd[:, 0:NL])
                    pack = spool.tile([128, 3 * NL], BF16, tag="pack")
                    hi_f = spool.tile([128, NL], F32, tag="hi_f")
                    r1 = spool.tile([128, NL], F32, tag="r1")
                    l1_f = spool.tile([128, NL], F32, tag="l1_f")
                    r2 = spool.tile([128, NL], F32, tag="r2")
                    hi_b = spool.tile([128, NL], BF16, tag="hi_b")
                    nc.vector.tensor_copy(hi_b[:, :], hcl[:, :])
                    nc.vector.tensor_copy(hi_f[:, :], hi_b[:, :])
                    nc.vector.tensor_sub(r1[:, :], hcl[:, :], hi_f[:, :])
                    nc.vector.tensor_copy(pack[:, NL:2 * NL], r1[:, :])
                    nc.vector.tensor_copy(l1_f[:, :], pack[:, NL:2 * NL])
                    nc.vector.tensor_sub(r2[:, :], r1[:, :], l1_f[:, :])
                    nc.vector.tensor_copy(pack[:, 2 * NL:3 * NL], r2[:, :])
                    nc.scalar.activation(pack[:, 0:NL], hi_b[:, :],
                                         mybir.ActivationFunctionType.Identity,
                                         bias=negm[:, :])

                    nc.sync.dma_start(
                        AP(scr_d[:, :, :].tensor, b * 3 * LNL,
                           [[NL, 128], [LNL, 3], [1, NL]]),
                        pack[:, :])
                    hl = hlpool.tile([3, LNL], BF16, tag="hl")
                    nc.sync.dma_start(hl[:, :],
                                      AP(scr_d[:, :, :].tensor, b * 3 * LNL,
                                         [[LNL, 3], [1, LNL]]))

                    out_sb = opool.tile([128, LNL], F32, tag="out")
                    for ci in range(4):
                        t_c = pspool.tile([128, 2048], F32, tag="ps")
                        for q in range(4):
                            col = ci * 2048 + q * 512
                            nc.tensor.matmul(t_c[:, q * 512:(q + 1) * 512],
                                             ones3[:, :],
                                             hl[:, col:col + 512],
                                             start=True, stop=True)
                        o_ap = AP(out_sb[:, :].tensor,
                                  out_sb[:, :].offset + ci * 2048,
                                  [list(out_sb[:, :].ap[0]), [NL, 32], [1, NL]])
                        p_ap = AP(t_c[:, :].tensor, t_c[:, :].offset,
                                  [list(t_c[:, :].ap[0]), [NL, 32], [1, NL]])
                        d_ap = AP(dep[:, :].tensor, dep[:, :].offset,
                                  [list(dep[:, :].ap[0]), [0, 32], [1, NL]])
                        nc.vector.tensor_tensor(o_ap, p_ap, d_ap,
                                                mybir.AluOpType.add)

                    nc.sync.dma_start(
                        AP(out_d[:, :, :].tensor, b * L * LNL,
                           [[LNL, 128], [1, LNL]]),
                        out_sb[:, :])
                    nc.sync.dma_start(
                        AP(out_d[:, :, :].tensor, b * L * LNL,
                           [[(L + 1) * NL, 128], [1, NL]]),
                        ninf[:, :])

    _split_excess_waits(nc)
    return nc


def make_in_maps(emb_sentences, att_sentences, W):
    emb = np.asarray(emb_sentences, dtype=np.float32)
    att = np.asarray(att_sentences)
    w = np.asarray(W, dtype=np.float32)
    embT = np.ascontiguousarray(emb.transpose(0, 2, 1))          # [B, D, L]
    w1T = np.ascontiguousarray(w[:, :D].T)                       # [D, NL]
    w2T = np.ascontiguousarray(w[:, D:].T)                       # [D, NL]
    negm = np.where(att, 0.0, NEG_INF).astype(np.float32)[:, :, None]
    return [{
        "embT": np.ascontiguousarray(embT[c * NB:(c + 1) * NB]),
        "w1T": w1T,
        "w2T": w2T,
        "negm": np.ascontiguousarray(negm[c * NB:(c + 1) * NB]),
    } for c in range(NCORES)]


_NC_CACHE = {}


def kernel(emb_sentences, att_sentences, W):
    if "nc" not in _NC_CACHE:
        _NC_CACHE["nc"] = build_nc()
    nc = _NC_CACHE["nc"]
    in_maps = make_in_maps(emb_sentences, att_sentences, W)
    res = bass_utils.run_bass_kernel_spmd(nc, in_maps, core_ids=list(range(NCORES)))
    return np.concatenate([res.results[c]["out"] for c in range(NCORES)], axis=0)
